# revision 1
# baseline (speedup 1.0000x reference)
"""Trainium2 Bass kernel for DepthWiseSeparableConv (shared-3x3 dw conv + BN+ReLU + 1x1 conv + BN+ReLU).

Strategy (8 NeuronCores, data-parallel over batch N=32 -> 4 images/core):
  - Load x [c,hw] (cast fp32->bf16 during SWDGE DMA).
  - PE transpose (regular matmul vs identity) to x^T chunks [112 pix, c].
  - Depthwise 3x3 conv as banded matmuls: out[c, pix-chunk] accumulated in
    PSUM from 3 banded matrices B_{-1},B_0,B_{+1} (stationary = x^T chunk,
    moving = banded matrix) -> output lands directly in [c, pix] layout.
  - BN1+ReLU fused into one ScalarE activation (per-partition scale/bias),
    cast to bf16 -> y [c, pix].
  - 1x1 conv = GEMM: psum[o, pix] += W^T[c,o].T @ y[c,pix] over 2 c-tiles.
  - BN2+ReLU fused into ScalarE activation -> z fp32 -> DMA out.

Self-contained: hardcodes all shapes; no file reads.
"""

import numpy as np

N, C, CO, H, W = 32, 256, 512, 56, 56
EPS = 1e-5
N_CORES = 8
NPC = N // N_CORES      # images per core
HW = H * W              # 3136
CH = 112                # pixel chunk = 2 rows of 56
NCHUNK = HW // CH       # 28
NPAIR = NCHUNK // 2     # 14
CT = C // 128           # 2 c-tiles
OT = CO // 128          # 4 o-tiles
GN = 448                # gemm pixel-block
NPB = HW // GN          # 7

_cache = {}


def _build_program():
    import concourse.mybir as mybir
    import concourse.tile as tile
    from concourse import bacc

    f32 = mybir.dt.float32
    bf16 = mybir.dt.bfloat16

    nc = bacc.Bacc("TRN2", target_bir_lowering=False, debug=False)

    x_d = nc.dram_tensor("x", [NPC, C, H, W], f32, kind="ExternalInput").ap()
    bmat_d = nc.dram_tensor("bmat", [CH, 3 * CH], bf16, kind="ExternalInput").ap()
    ident_d = nc.dram_tensor("ident", [128, 128], bf16, kind="ExternalInput").ap()
    wT_d = nc.dram_tensor("wT", [C, CO], bf16, kind="ExternalInput").ap()
    s1_d = nc.dram_tensor("s1", [128, CT], f32, kind="ExternalInput").ap()
    t1_d = nc.dram_tensor("t1", [128, CT], f32, kind="ExternalInput").ap()
    s2_d = nc.dram_tensor("s2", [128, OT], f32, kind="ExternalInput").ap()
    t2_d = nc.dram_tensor("t2", [128, OT], f32, kind="ExternalInput").ap()
    z_d = nc.dram_tensor("z", [NPC, CO, H, W], f32, kind="ExternalOutput").ap()

    relu = mybir.ActivationFunctionType.Relu

    with tile.TileContext(nc) as tc:
        with (
            tc.tile_pool(name="singles", bufs=1) as singles,
            tc.tile_pool(name="xp", bufs=4) as xp,
            tc.tile_pool(name="xtp", bufs=2) as xtp,
            tc.tile_pool(name="yp", bufs=4) as yp,
            tc.tile_pool(name="zp", bufs=3) as zp,
            tc.tile_pool(name="tps", bufs=2, space="PSUM") as tps,
            tc.tile_pool(name="cvs", bufs=4, space="PSUM") as cvs,
            tc.tile_pool(name="zps", bufs=2, space="PSUM") as zps,
        ):
            bmat_sb = singles.tile([CH, 3 * CH], bf16)
            nc.sync.dma_start(out=bmat_sb, in_=bmat_d)
            ident_sb = singles.tile([128, 128], bf16)
            nc.sync.dma_start(out=ident_sb, in_=ident_d)
            w_sb = singles.tile([128, CT, CO], bf16)
            for ci in range(CT):
                nc.sync.dma_start(out=w_sb[:, ci, :], in_=wT_d[128 * ci:128 * (ci + 1), :])
            s1_sb = singles.tile([128, CT], f32)
            nc.sync.dma_start(out=s1_sb, in_=s1_d)
            t1_sb = singles.tile([128, CT], f32)
            nc.sync.dma_start(out=t1_sb, in_=t1_d)
            s2_sb = singles.tile([128, OT], f32)
            nc.sync.dma_start(out=s2_sb, in_=s2_d)
            t2_sb = singles.tile([128, OT], f32)
            nc.sync.dma_start(out=t2_sb, in_=t2_d)

            for img in range(NPC):
                # ---- stage A: load x, cast fp32->bf16 during DMA ----
                x_sb = xp.tile([128, CT, HW], bf16, tag="x")
                for ci in range(CT):
                    nc.gpsimd.dma_start(
                        out=x_sb[:, ci, :],
                        in_=x_d[img, 128 * ci:128 * (ci + 1), :, :].rearrange("c h w -> c (h w)"),
                    )

                # ---- stage B: transpose to x^T chunks [112, c] ----
                # xt_sb cols: chunk s, ctile ci at s*256 + ci*128
                xt_sb = xtp.tile([CH, NCHUNK * C], bf16, tag="xt")
                for q in range(NPAIR):
                    t_ps = tps.tile([CH, 512], f32, tag="tps")
                    for k in range(2):          # chunk s = 2q+k
                        s = 2 * q + k
                        for ci in range(CT):
                            nc.tensor.matmul(
                                t_ps[:, 256 * k + 128 * ci: 256 * k + 128 * (ci + 1)],
                                lhsT=x_sb[:, ci, CH * s: CH * (s + 1)],
                                rhs=ident_sb,
                                start=True, stop=True,
                            )
                    nc.vector.tensor_copy(xt_sb[:, 512 * q: 512 * (q + 1)], t_ps)

                def xt(s, ci):
                    return xt_sb[:, 256 * s + 128 * ci: 256 * s + 128 * (ci + 1)]

                # ---- stage C: depthwise conv as banded matmuls + BN1+ReLU ----
                y_sb = yp.tile([128, CT, HW], bf16, tag="y")
                for ci in range(CT):
                    for mp in range(NPAIR):
                        m = 2 * mp
                        cv = cvs.tile([128, 2 * CH], f32, tag="cv")
                        # sources m, m+1 contribute full 224; m-1, m+2 edges
                        mms = [(m, bmat_sb[:, CH:3 * CH], 0, 2 * CH)]
                        mms.append((m + 1, bmat_sb[:, 0:2 * CH], 0, 2 * CH))
                        if m - 1 >= 0:
                            mms.append((m - 1, bmat_sb[:, 2 * CH:3 * CH], 0, CH))
                        if m + 2 < NCHUNK:
                            mms.append((m + 2, bmat_sb[:, 0:CH], CH, 2 * CH))
                        for i, (s, rhs, lo, hi) in enumerate(mms):
                            nc.tensor.matmul(
                                cv[:, lo:hi], lhsT=xt(s, ci), rhs=rhs,
                                start=(i == 0), stop=(i == len(mms) - 1),
                                skip_group_check=True,
                            )
                        nc.scalar.activation(
                            out=y_sb[:, ci, 2 * CH * mp: 2 * CH * (mp + 1)],
                            in_=cv, func=relu,
                            scale=s1_sb[:, ci:ci + 1], bias=t1_sb[:, ci:ci + 1],
                        )

                # ---- stage D: pointwise GEMM + BN2+ReLU ----
                for oi in range(OT):
                    z_sb = zp.tile([128, HW], f32, tag="z")
                    for pb in range(NPB):
                        z_ps = zps.tile([128, GN], f32, tag="zps")
                        for ci in range(CT):
                            nc.tensor.matmul(
                                z_ps,
                                lhsT=w_sb[:, ci, 128 * oi:128 * (oi + 1)],
                                rhs=y_sb[:, ci, GN * pb: GN * (pb + 1)],
                                start=(ci == 0), stop=(ci == CT - 1),
                            )
                        nc.scalar.activation(
                            out=z_sb[:, GN * pb: GN * (pb + 1)],
                            in_=z_ps, func=relu,
                            scale=s2_sb[:, oi:oi + 1], bias=t2_sb[:, oi:oi + 1],
                        )
                    # ---- stage E: DMA out ----
                    nc.sync.dma_start(
                        out=z_d[img, 128 * oi:128 * (oi + 1), :, :].rearrange("o h w -> o (h w)"),
                        in_=z_sb,
                    )

    nc.compile()
    return nc


def _build_bmats(k2d):
    """B[t][p,j] = k2d[1+dh, 1+dw] with dh = 2t + p//56 - j//56, dw = p%56 - j%56."""
    p = np.arange(CH)
    j = np.arange(CH)
    ph, pw = p // W, p % W
    jh, jw = j // W, j % W
    out = []
    for t in (1, 0, -1):  # concat order [B_{+1} | B_0 | B_{-1}]
        dh = 2 * t + ph[:, None] - jh[None, :]
        dw = pw[:, None] - jw[None, :]
        ok = (np.abs(dh) <= 1) & (np.abs(dw) <= 1)
        B = np.where(ok, k2d[np.clip(1 + dh, 0, 2), np.clip(1 + dw, 0, 2)], 0.0)
        out.append(B)
    return np.concatenate(out, axis=1).astype(np.float32)  # [112, 336]


def kernel(x, w_dw, b_dw, bn1_gamma, bn1_beta, bn1_mean, bn1_var,
           w_pw, b_pw, bn2_gamma, bn2_beta, bn2_mean, bn2_var):
    import ml_dtypes
    from concourse import bass_utils

    bf = ml_dtypes.bfloat16

    if "nc" not in _cache:
        _cache["nc"] = _build_program()
    nc = _cache["nc"]

    x = np.asarray(x, np.float32)
    s1 = (bn1_gamma / np.sqrt(bn1_var + EPS)).astype(np.float32)
    t1 = (bn1_beta - bn1_mean * s1 + s1 * float(np.asarray(b_dw).reshape(-1)[0])).astype(np.float32)
    s2 = (bn2_gamma / np.sqrt(bn2_var + EPS)).astype(np.float32)
    t2 = (bn2_beta - bn2_mean * s2 + s2 * np.asarray(b_pw, np.float32)).astype(np.float32)

    bmat = _build_bmats(np.asarray(w_dw, np.float32)[0, 0]).astype(bf)
    ident = np.eye(128, dtype=bf)
    wT = np.ascontiguousarray(np.asarray(w_pw, np.float32).T).astype(bf)  # [C, CO]

    shared = {
        "bmat": bmat,
        "ident": ident,
        "wT": wT,
        "s1": np.ascontiguousarray(s1.reshape(CT, 128).T),
        "t1": np.ascontiguousarray(t1.reshape(CT, 128).T),
        "s2": np.ascontiguousarray(s2.reshape(OT, 128).T),
        "t2": np.ascontiguousarray(t2.reshape(OT, 128).T),
    }
    in_maps = [{"x": np.ascontiguousarray(x[NPC * i: NPC * (i + 1)]), **shared}
               for i in range(N_CORES)]

    res = bass_utils.run_bass_kernel_spmd(nc, in_maps, core_ids=list(range(N_CORES)))
    z = np.concatenate([res.results[i]["z"] for i in range(N_CORES)], axis=0)
    return np.asarray(z, np.float32)


# revision 6
# speedup vs baseline: 249.1608x; 249.1608x over previous
"""Trainium2 Bass kernel for DepthWiseSeparableConv (shared-3x3 dw conv + BN+ReLU + 1x1 conv + BN+ReLU).

Strategy (8 NeuronCores, data-parallel over batch N=32 -> 4 images/core):
  - Load x [c,hw] (cast fp32->bf16 during SWDGE DMA).
  - PE transpose (regular matmul vs identity) to x^T chunks [112 pix, c].
  - Depthwise 3x3 conv as banded matmuls: out[c, pix-chunk] accumulated in
    PSUM from 3 banded matrices B_{-1},B_0,B_{+1} (stationary = x^T chunk,
    moving = banded matrix) -> output lands directly in [c, pix] layout.
  - BN1+ReLU fused into one ScalarE activation (per-partition scale/bias),
    cast to bf16 -> y [c, pix].
  - 1x1 conv = GEMM: psum[o, pix] += W^T[c,o].T @ y[c,pix] over 2 c-tiles.
  - BN2+ReLU fused into ScalarE activation -> z fp32 -> DMA out.

Self-contained: hardcodes all shapes; no file reads.
"""

import numpy as np

N, C, CO, H, W = 32, 256, 512, 56, 56
EPS = 1e-5
N_CORES = 8
NPC = N // N_CORES      # images per core
HW = H * W              # 3136
CH = 112                # pixel chunk = 2 rows of 56
NCHUNK = HW // CH       # 28
NPAIR = NCHUNK // 2     # 14
CT = C // 128           # 2 c-tiles
OT = CO // 128          # 4 o-tiles
GN = 448                # gemm pixel-block
NPB = HW // GN          # 7

_cache = {}


def _build_program():
    import concourse.mybir as mybir
    import concourse.tile as tile
    from concourse import bacc

    f32 = mybir.dt.float32
    bf16 = mybir.dt.bfloat16

    nc = bacc.Bacc("TRN2", target_bir_lowering=False, debug=False)

    x_d = nc.dram_tensor("x", [NPC, C, H, W], f32, kind="ExternalInput").ap()
    bmat_d = nc.dram_tensor("bmat", [CH, 3 * CH], bf16, kind="ExternalInput").ap()
    ident_d = nc.dram_tensor("ident", [128, 128], bf16, kind="ExternalInput").ap()
    wT_d = nc.dram_tensor("wT", [C, CO], bf16, kind="ExternalInput").ap()
    s1_d = nc.dram_tensor("s1", [128, CT], f32, kind="ExternalInput").ap()
    t1_d = nc.dram_tensor("t1", [128, CT], f32, kind="ExternalInput").ap()
    s2_d = nc.dram_tensor("s2", [128, OT], f32, kind="ExternalInput").ap()
    t2_d = nc.dram_tensor("t2", [128, OT], f32, kind="ExternalInput").ap()
    z_d = nc.dram_tensor("z", [NPC, CO, H, W], f32, kind="ExternalOutput").ap()

    relu = mybir.ActivationFunctionType.Relu

    with tile.TileContext(nc) as tc:
        with (
            tc.tile_pool(name="singles", bufs=1) as singles,
            tc.tile_pool(name="xp", bufs=4) as xp,
            tc.tile_pool(name="xtp", bufs=2) as xtp,
            tc.tile_pool(name="yp", bufs=4) as yp,
            tc.tile_pool(name="zp", bufs=3) as zp,
            tc.tile_pool(name="tps", bufs=2, space="PSUM") as tps,
            tc.tile_pool(name="cvs", bufs=4, space="PSUM") as cvs,
            tc.tile_pool(name="zps", bufs=2, space="PSUM") as zps,
        ):
            bmat_sb = singles.tile([CH, 3 * CH], bf16)
            nc.sync.dma_start(out=bmat_sb, in_=bmat_d)
            ident_sb = singles.tile([128, 128], bf16)
            nc.sync.dma_start(out=ident_sb, in_=ident_d)
            w_sb = singles.tile([128, CT, CO], bf16)
            for ci in range(CT):
                nc.sync.dma_start(out=w_sb[:, ci, :], in_=wT_d[128 * ci:128 * (ci + 1), :])
            s1_sb = singles.tile([128, CT], f32)
            nc.sync.dma_start(out=s1_sb, in_=s1_d)
            t1_sb = singles.tile([128, CT], f32)
            nc.sync.dma_start(out=t1_sb, in_=t1_d)
            s2_sb = singles.tile([128, OT], f32)
            nc.sync.dma_start(out=s2_sb, in_=s2_d)
            t2_sb = singles.tile([128, OT], f32)
            nc.sync.dma_start(out=t2_sb, in_=t2_d)

            for img in range(NPC):
                # ---- stage A: load x, cast fp32->bf16 during DMA ----
                x_sb = xp.tile([128, CT, HW], bf16, tag="x")
                for ci in range(CT):
                    nc.gpsimd.dma_start(
                        out=x_sb[:, ci, :],
                        in_=x_d[img, 128 * ci:128 * (ci + 1), :, :].rearrange("c h w -> c (h w)"),
                    )

                # ---- stage B: transpose to x^T chunks [112, c] ----
                # xt_sb cols: chunk s, ctile ci at s*256 + ci*128
                xt_sb = xtp.tile([CH, NCHUNK * C], bf16, tag="xt")
                for q in range(NPAIR):
                    t_ps = tps.tile([CH, 512], f32, tag="tps")
                    for k in range(2):          # chunk s = 2q+k
                        s = 2 * q + k
                        for ci in range(CT):
                            nc.tensor.matmul(
                                t_ps[:, 256 * k + 128 * ci: 256 * k + 128 * (ci + 1)],
                                lhsT=x_sb[:, ci, CH * s: CH * (s + 1)],
                                rhs=ident_sb,
                                start=True, stop=True,
                            )
                    nc.vector.tensor_copy(xt_sb[:, 512 * q: 512 * (q + 1)], t_ps)

                def xt(s, ci):
                    return xt_sb[:, 256 * s + 128 * ci: 256 * s + 128 * (ci + 1)]

                # ---- stage C: depthwise conv as banded matmuls + BN1+ReLU ----
                y_sb = yp.tile([128, CT, HW], bf16, tag="y")
                for ci in range(CT):
                    for mp in range(NPAIR):
                        m = 2 * mp
                        cv = cvs.tile([128, 2 * CH], f32, tag="cv")
                        # sources m, m+1 contribute full 224; m-1, m+2 edges
                        mms = [(m, bmat_sb[:, CH:3 * CH], 0, 2 * CH)]
                        mms.append((m + 1, bmat_sb[:, 0:2 * CH], 0, 2 * CH))
                        if m - 1 >= 0:
                            mms.append((m - 1, bmat_sb[:, 2 * CH:3 * CH], 0, CH))
                        if m + 2 < NCHUNK:
                            mms.append((m + 2, bmat_sb[:, 0:CH], CH, 2 * CH))
                        for i, (s, rhs, lo, hi) in enumerate(mms):
                            nc.tensor.matmul(
                                cv[:, lo:hi], lhsT=xt(s, ci), rhs=rhs,
                                start=(i == 0), stop=(i == len(mms) - 1),
                                skip_group_check=True,
                            )
                        nc.scalar.activation(
                            out=y_sb[:, ci, 2 * CH * mp: 2 * CH * (mp + 1)],
                            in_=cv, func=relu,
                            scale=s1_sb[:, ci:ci + 1], bias=t1_sb[:, ci:ci + 1],
                        )

                # ---- stage D: pointwise GEMM + BN2+ReLU ----
                for oi in range(OT):
                    z_sb = zp.tile([128, HW], f32, tag="z")
                    for pb in range(NPB):
                        z_ps = zps.tile([128, GN], f32, tag="zps")
                        for ci in range(CT):
                            nc.tensor.matmul(
                                z_ps,
                                lhsT=w_sb[:, ci, 128 * oi:128 * (oi + 1)],
                                rhs=y_sb[:, ci, GN * pb: GN * (pb + 1)],
                                start=(ci == 0), stop=(ci == CT - 1),
                            )
                        nc.scalar.activation(
                            out=z_sb[:, GN * pb: GN * (pb + 1)],
                            in_=z_ps, func=relu,
                            scale=s2_sb[:, oi:oi + 1], bias=t2_sb[:, oi:oi + 1],
                        )
                    # ---- stage E: DMA out ----
                    nc.sync.dma_start(
                        out=z_d[img, 128 * oi:128 * (oi + 1), :, :].rearrange("o h w -> o (h w)"),
                        in_=z_sb,
                    )

    nc.compile()
    return nc


def _build_bmats(k2d):
    """B[t][p,j] = k2d[1+dh, 1+dw] with dh = 2t + p//56 - j//56, dw = p%56 - j%56."""
    p = np.arange(CH)
    j = np.arange(CH)
    ph, pw = p // W, p % W
    jh, jw = j // W, j % W
    out = []
    for t in (1, 0, -1):  # concat order [B_{+1} | B_0 | B_{-1}]
        dh = 2 * t + ph[:, None] - jh[None, :]
        dw = pw[:, None] - jw[None, :]
        ok = (np.abs(dh) <= 1) & (np.abs(dw) <= 1)
        B = np.where(ok, k2d[np.clip(1 + dh, 0, 2), np.clip(1 + dw, 0, 2)], 0.0)
        out.append(B)
    return np.concatenate(out, axis=1).astype(np.float32)  # [112, 336]


def _get_runner(nc):
    """Build a cached jitted shard_map executable mirroring
    concourse.bass2jax.run_bass_via_pjrt (which re-traces on every call)."""
    import jax
    import jax.numpy as jnp
    import concourse.mybir as mybir
    from jax.sharding import Mesh, PartitionSpec
    from jax.experimental.shard_map import shard_map
    from concourse.bass2jax import (
        _bass_exec_p, install_neuronx_cc_hook, partition_id_tensor)

    install_neuronx_cc_hook()

    partition_name = nc.partition_id_tensor.name if nc.partition_id_tensor else None

    in_names, out_names, out_avals = [], [], []
    for alloc in nc.m.functions[0].allocations:
        if not isinstance(alloc, mybir.MemoryLocationSet):
            continue
        name = alloc.memorylocations[0].name
        if alloc.kind == "ExternalInput":
            if name != partition_name:
                in_names.append(name)
        elif alloc.kind == "ExternalOutput":
            out_names.append(name)
            out_avals.append(jax.core.ShapedArray(
                tuple(alloc.tensor_shape), mybir.dt.np(alloc.dtype)))
    n_params = len(in_names)
    all_names = list(in_names) + list(out_names)
    if partition_name is not None:
        all_names.append(partition_name)

    def _body(*args):
        operands = list(args)
        if partition_name is not None:
            operands.append(partition_id_tensor())
        return tuple(_bass_exec_p.bind(
            *operands,
            out_avals=tuple(out_avals),
            in_names=tuple(all_names),
            out_names=tuple(out_names),
            lowering_input_output_aliases=(),
            sim_require_finite=True,
            sim_require_nnan=True,
            nc=nc,
        ))

    n_outs = len(out_avals)
    devices = jax.devices()[:N_CORES]
    mesh = Mesh(np.asarray(devices), ("core",))
    fn = jax.jit(
        shard_map(
            _body, mesh=mesh,
            in_specs=(PartitionSpec("core"),) * (n_params + n_outs),
            out_specs=(PartitionSpec("core"),) * len(out_names),
            check_rep=False,
        ),
        donate_argnums=tuple(range(n_params, n_params + n_outs)),
        keep_unused=True,
    )
    out_shapes = [(N_CORES * a.shape[0], *a.shape[1:]) for a in out_avals]
    out_dtypes = [a.dtype for a in out_avals]
    return fn, in_names, out_names, out_shapes, out_dtypes


def _prep_inputs(x, w_dw, b_dw, bn1_gamma, bn1_beta, bn1_mean, bn1_var,
                 w_pw, b_pw, bn2_gamma, bn2_beta, bn2_mean, bn2_var):
    import ml_dtypes
    bf = ml_dtypes.bfloat16

    x = np.asarray(x, np.float32)
    s1 = (bn1_gamma / np.sqrt(bn1_var + EPS)).astype(np.float32)
    t1 = (bn1_beta - bn1_mean * s1 + s1 * float(np.asarray(b_dw).reshape(-1)[0])).astype(np.float32)
    s2 = (bn2_gamma / np.sqrt(bn2_var + EPS)).astype(np.float32)
    t2 = (bn2_beta - bn2_mean * s2 + s2 * np.asarray(b_pw, np.float32)).astype(np.float32)

    shared = {
        "bmat": _build_bmats(np.asarray(w_dw, np.float32)[0, 0]).astype(bf),
        "ident": np.eye(128, dtype=bf),
        "wT": np.ascontiguousarray(np.asarray(w_pw, np.float32).T).astype(bf),
        "s1": np.ascontiguousarray(s1.reshape(CT, 128).T),
        "t1": np.ascontiguousarray(t1.reshape(CT, 128).T),
        "s2": np.ascontiguousarray(s2.reshape(OT, 128).T),
        "t2": np.ascontiguousarray(t2.reshape(OT, 128).T),
    }
    return [{"x": np.ascontiguousarray(x[NPC * i: NPC * (i + 1)]), **shared}
            for i in range(N_CORES)]


def kernel(x, w_dw, b_dw, bn1_gamma, bn1_beta, bn1_mean, bn1_var,
           w_pw, b_pw, bn2_gamma, bn2_beta, bn2_mean, bn2_var):
    if "nc" not in _cache:
        _cache["nc"] = _build_program()
    nc = _cache["nc"]

    in_maps = _prep_inputs(x, w_dw, b_dw, bn1_gamma, bn1_beta, bn1_mean, bn1_var,
                           w_pw, b_pw, bn2_gamma, bn2_beta, bn2_mean, bn2_var)
    try:
        if "runner" not in _cache:
            _cache["runner"] = _get_runner(nc)
        fn, in_names, out_names, out_shapes, out_dtypes = _cache["runner"]
        concat_in = [np.concatenate([m[name] for m in in_maps], axis=0)
                     for name in in_names]
        zeros = [np.zeros(s, d) for s, d in zip(out_shapes, out_dtypes)]
        outs = fn(*concat_in, *zeros)
        z = np.asarray(outs[out_names.index("z")])
        return z.astype(np.float32)
    except Exception:
        from concourse import bass_utils
        res = bass_utils.run_bass_kernel_spmd(nc, in_maps, core_ids=list(range(N_CORES)))
        z = np.concatenate([res.results[i]["z"] for i in range(N_CORES)], axis=0)
        return np.asarray(z, np.float32)


def _kernel_spmd_reference_path(x, w_dw, b_dw, bn1_gamma, bn1_beta, bn1_mean, bn1_var,
                                w_pw, b_pw, bn2_gamma, bn2_beta, bn2_mean, bn2_var):
    """Original path through bass_utils.run_bass_kernel_spmd (kept for checking)."""
    from concourse import bass_utils

    if "nc" not in _cache:
        _cache["nc"] = _build_program()
    nc = _cache["nc"]

    in_maps = _prep_inputs(x, w_dw, b_dw, bn1_gamma, bn1_beta, bn1_mean, bn1_var,
                           w_pw, b_pw, bn2_gamma, bn2_beta, bn2_mean, bn2_var)
    res = bass_utils.run_bass_kernel_spmd(nc, in_maps, core_ids=list(range(N_CORES)))
    z = np.concatenate([res.results[i]["z"] for i in range(N_CORES)], axis=0)
    return np.asarray(z, np.float32)


# revision 10
# speedup vs baseline: 257.1131x; 1.0319x over previous
"""Trainium2 Bass kernel for DepthWiseSeparableConv (shared-3x3 dw conv + BN+ReLU + 1x1 conv + BN+ReLU).

Strategy (8 NeuronCores, data-parallel over batch N=32 -> 4 images/core):
  - Load x [c,hw] (cast fp32->bf16 during SWDGE DMA).
  - PE transpose (regular matmul vs identity) to x^T chunks [112 pix, c].
  - Depthwise 3x3 conv as banded matmuls: out[c, pix-chunk] accumulated in
    PSUM from 3 banded matrices B_{-1},B_0,B_{+1} (stationary = x^T chunk,
    moving = banded matrix) -> output lands directly in [c, pix] layout.
  - BN1+ReLU fused into one ScalarE activation (per-partition scale/bias),
    cast to bf16 -> y [c, pix].
  - 1x1 conv = GEMM: psum[o, pix] += W^T[c,o].T @ y[c,pix] over 2 c-tiles.
  - BN2+ReLU fused into ScalarE activation -> z fp32 -> DMA out.

Self-contained: hardcodes all shapes; no file reads.
"""

import numpy as np

N, C, CO, H, W = 32, 256, 512, 56, 56
EPS = 1e-5
N_CORES = 8
NPC = N // N_CORES      # images per core
HW = H * W              # 3136
CH = 112                # pixel chunk = 2 rows of 56
NCHUNK = HW // CH       # 28
NPAIR = NCHUNK // 2     # 14
CT = C // 128           # 2 c-tiles
OT = CO // 128          # 4 o-tiles
GN = 448                # gemm pixel-block
NPB = HW // GN          # 7

_cache = {}


def _build_program():
    import concourse.mybir as mybir
    import concourse.tile as tile
    from concourse import bacc

    f32 = mybir.dt.float32
    bf16 = mybir.dt.bfloat16

    nc = bacc.Bacc("TRN2", target_bir_lowering=False, debug=False)

    x_d = nc.dram_tensor("x", [NPC, C, H, W], f32, kind="ExternalInput").ap()
    bmat_d = nc.dram_tensor("bmat", [CH, 3 * CH], bf16, kind="ExternalInput").ap()
    # block-diagonal scale: sdiag[:, ci*128:(ci+1)*128] = diag(s1[ci-tile])
    sdiag_d = nc.dram_tensor("sdiag", [128, CT * 128], bf16, kind="ExternalInput").ap()
    wT_d = nc.dram_tensor("wT", [C, CO], bf16, kind="ExternalInput").ap()  # s2-folded
    t1_d = nc.dram_tensor("t1", [128, CT], f32, kind="ExternalInput").ap()
    t2_d = nc.dram_tensor("t2", [128, OT], f32, kind="ExternalInput").ap()
    z_d = nc.dram_tensor("z", [NPC, CO, H, W], f32, kind="ExternalOutput").ap()

    relu = mybir.ActivationFunctionType.Relu
    add = mybir.AluOpType.add
    amax = mybir.AluOpType.max

    with tile.TileContext(nc) as tc:
        with (
            tc.tile_pool(name="singles", bufs=1) as singles,
            tc.tile_pool(name="xp", bufs=3) as xp,
            tc.tile_pool(name="xtp", bufs=2) as xtp,
            tc.tile_pool(name="yp", bufs=2) as yp,
            tc.tile_pool(name="zp", bufs=5) as zp,
            tc.tile_pool(name="tps", bufs=2, space="PSUM") as tps,
            tc.tile_pool(name="cvs", bufs=2, space="PSUM") as cvs,
            tc.tile_pool(name="zps", bufs=2, space="PSUM") as zps,
        ):
            bmat_sb = singles.tile([CH, 3 * CH], bf16)
            nc.sync.dma_start(out=bmat_sb, in_=bmat_d)
            sdiag_sb = singles.tile([128, CT * 128], bf16)
            nc.sync.dma_start(out=sdiag_sb, in_=sdiag_d)
            w_sb = singles.tile([128, CT, CO], bf16)
            for ci in range(CT):
                nc.sync.dma_start(out=w_sb[:, ci, :], in_=wT_d[128 * ci:128 * (ci + 1), :])
            t1_sb = singles.tile([128, CT], f32)
            nc.sync.dma_start(out=t1_sb, in_=t1_d)
            t2_sb = singles.tile([128, OT], f32)
            nc.sync.dma_start(out=t2_sb, in_=t2_d)

            # epilogue helper: out = relu(in + bias[p]) on alternating engines
            epi_ctr = [0]

            def epilogue(out_ap, in_ap, bias_ap):
                use_act = (epi_ctr[0] % 2 == 0)
                epi_ctr[0] += 1
                if bias_ap is None:
                    if use_act:
                        nc.scalar.copy(out=out_ap, in_=in_ap)
                    else:
                        nc.vector.tensor_copy(out_ap, in_ap)
                elif use_act:
                    nc.scalar.activation(out=out_ap, in_=in_ap, func=relu,
                                         bias=bias_ap, scale=1.0)
                else:
                    nc.vector.tensor_scalar(out=out_ap, in0=in_ap,
                                            scalar1=bias_ap, scalar2=0.0,
                                            op0=add, op1=amax)

            # conv moving-operand slices of bmat = [B+1 | B0 | B-1]
            A_even = bmat_sb[:, CH:3 * CH]       # [B0 | B-1]
            A_odd = bmat_sb[:, 0:2 * CH]         # [B+1 | B0]
            B_plus = bmat_sb[:, 0:CH]            # B+1 (from even source s -> chunk s-1)
            B_minus = bmat_sb[:, 2 * CH:3 * CH]  # B-1 (from odd source s -> chunk s+1)

            for img in range(NPC):
                # ---- stage A: load x, cast fp32->bf16 during DMA ----
                x_sb = xp.tile([128, CT, HW], bf16, tag="x")
                for ci in range(CT):
                    nc.gpsimd.dma_start(
                        out=x_sb[:, ci, :],
                        in_=x_d[img, 128 * ci:128 * (ci + 1), :, :].rearrange("c h w -> c (h w)"),
                    )

                # ---- stage B: transpose (x^T scaled by s1 via block-diag rhs) ----
                xt_sb = xtp.tile([CH, NCHUNK * C], bf16, tag="xt")
                for q in range(NPAIR):
                    t_ps = tps.tile([CH, 512], f32, tag="tps")
                    for k in range(2):          # chunk s = 2q+k
                        s = 2 * q + k
                        for ci in range(CT):
                            nc.tensor.matmul(
                                t_ps[:, 256 * k + 128 * ci: 256 * k + 128 * (ci + 1)],
                                lhsT=x_sb[:, ci, CH * s: CH * (s + 1)],
                                rhs=sdiag_sb[:, 128 * ci:128 * (ci + 1)],
                                start=True, stop=True,
                            )
                    epilogue(xt_sb[:, 512 * q: 512 * (q + 1)], t_ps, None)

                def xt(s, ci):
                    return xt_sb[:, 256 * s + 128 * ci: 256 * s + 128 * (ci + 1)]

                # ---- stage C: depthwise conv, scatter form over sources ----
                # combined psum tile g covers chunks 4g..4g+3 as [128, bank 2, 256]
                y_sb = yp.tile([128, CT, HW], bf16, tag="y")
                NG = NCHUNK // 4  # 7
                for ci in range(CT):
                    for g in range(NG):
                        # each [*, q, :] half is a full 2 KiB PSUM bank (512 f32);
                        # only cols 0:224 are used. Separate banks so per-bank
                        # start=True clears cannot disturb the sibling pair.
                        cv = cvs.tile([128, 2, 512], f32, tag="cv")
                        mms = []  # (src_chunk, rhs, bank, lo, hi)
                        if g > 0:
                            mms.append((4 * g - 1, B_minus, 0, 0, CH))
                        mms.append((4 * g, A_even, 0, 0, 2 * CH))
                        mms.append((4 * g + 1, A_odd, 0, 0, 2 * CH))
                        mms.append((4 * g + 1, B_minus, 1, 0, CH))
                        mms.append((4 * g + 2, A_even, 1, 0, 2 * CH))
                        mms.append((4 * g + 2, B_plus, 0, CH, 2 * CH))
                        mms.append((4 * g + 3, A_odd, 1, 0, 2 * CH))
                        if g < NG - 1:
                            mms.append((4 * g + 4, B_plus, 1, CH, 2 * CH))
                        first = {0: True, 1: True}
                        last_idx = {0: -1, 1: -1}
                        for i, (_, _, bank, _, _) in enumerate(mms):
                            last_idx[bank] = i
                        for i, (s, rhs, bank, lo, hi) in enumerate(mms):
                            nc.tensor.matmul(
                                cv[:, bank, lo:hi], lhsT=xt(s, ci), rhs=rhs,
                                start=first[bank], stop=(i == last_idx[bank]),
                                skip_group_check=True,
                            )
                            first[bank] = False
                        epilogue(
                            y_sb[:, ci, 448 * g: 448 * (g + 1)].rearrange(
                                "p (a b) -> p a b", a=2),
                            cv[:, :, 0:2 * CH],
                            t1_sb[:, ci:ci + 1],
                        )

                # ---- stage D: pointwise GEMM + BN2+ReLU (s2 folded into W) ----
                for oi in range(OT):
                    z_sb = zp.tile([128, HW], f32, tag="z")
                    for pb in range(NPB):
                        z_ps = zps.tile([128, GN], f32, tag="zps")
                        for ci in range(CT):
                            nc.tensor.matmul(
                                z_ps,
                                lhsT=w_sb[:, ci, 128 * oi:128 * (oi + 1)],
                                rhs=y_sb[:, ci, GN * pb: GN * (pb + 1)],
                                start=(ci == 0), stop=(ci == CT - 1),
                            )
                        epilogue(z_sb[:, GN * pb: GN * (pb + 1)], z_ps,
                                 t2_sb[:, oi:oi + 1])
                    # ---- stage E: DMA out ----
                    nc.sync.dma_start(
                        out=z_d[img, 128 * oi:128 * (oi + 1), :, :].rearrange("o h w -> o (h w)"),
                        in_=z_sb,
                    )

    nc.compile()
    return nc


def _build_bmats(k2d):
    """B[t][p,j] = k2d[1+dh, 1+dw] with dh = 2t + p//56 - j//56, dw = p%56 - j%56."""
    p = np.arange(CH)
    j = np.arange(CH)
    ph, pw = p // W, p % W
    jh, jw = j // W, j % W
    out = []
    for t in (1, 0, -1):  # concat order [B_{+1} | B_0 | B_{-1}]
        dh = 2 * t + ph[:, None] - jh[None, :]
        dw = pw[:, None] - jw[None, :]
        ok = (np.abs(dh) <= 1) & (np.abs(dw) <= 1)
        B = np.where(ok, k2d[np.clip(1 + dh, 0, 2), np.clip(1 + dw, 0, 2)], 0.0)
        out.append(B)
    return np.concatenate(out, axis=1).astype(np.float32)  # [112, 336]


def _get_runner(nc):
    """Build a cached jitted shard_map executable mirroring
    concourse.bass2jax.run_bass_via_pjrt (which re-traces on every call)."""
    import jax
    import jax.numpy as jnp
    import concourse.mybir as mybir
    from jax.sharding import Mesh, PartitionSpec
    from jax.experimental.shard_map import shard_map
    from concourse.bass2jax import (
        _bass_exec_p, install_neuronx_cc_hook, partition_id_tensor)

    install_neuronx_cc_hook()

    partition_name = nc.partition_id_tensor.name if nc.partition_id_tensor else None

    in_names, out_names, out_avals = [], [], []
    for alloc in nc.m.functions[0].allocations:
        if not isinstance(alloc, mybir.MemoryLocationSet):
            continue
        name = alloc.memorylocations[0].name
        if alloc.kind == "ExternalInput":
            if name != partition_name:
                in_names.append(name)
        elif alloc.kind == "ExternalOutput":
            out_names.append(name)
            out_avals.append(jax.core.ShapedArray(
                tuple(alloc.tensor_shape), mybir.dt.np(alloc.dtype)))
    n_params = len(in_names)
    all_names = list(in_names) + list(out_names)
    if partition_name is not None:
        all_names.append(partition_name)

    def _body(*args):
        operands = list(args)
        if partition_name is not None:
            operands.append(partition_id_tensor())
        return tuple(_bass_exec_p.bind(
            *operands,
            out_avals=tuple(out_avals),
            in_names=tuple(all_names),
            out_names=tuple(out_names),
            lowering_input_output_aliases=(),
            sim_require_finite=True,
            sim_require_nnan=True,
            nc=nc,
        ))

    n_outs = len(out_avals)
    devices = jax.devices()[:N_CORES]
    mesh = Mesh(np.asarray(devices), ("core",))
    fn = jax.jit(
        shard_map(
            _body, mesh=mesh,
            in_specs=(PartitionSpec("core"),) * (n_params + n_outs),
            out_specs=(PartitionSpec("core"),) * len(out_names),
            check_rep=False,
        ),
        donate_argnums=tuple(range(n_params, n_params + n_outs)),
        keep_unused=True,
    )
    out_shapes = [(N_CORES * a.shape[0], *a.shape[1:]) for a in out_avals]
    out_dtypes = [a.dtype for a in out_avals]
    return fn, in_names, out_names, out_shapes, out_dtypes


def _prep_inputs(x, w_dw, b_dw, bn1_gamma, bn1_beta, bn1_mean, bn1_var,
                 w_pw, b_pw, bn2_gamma, bn2_beta, bn2_mean, bn2_var):
    import ml_dtypes
    bf = ml_dtypes.bfloat16

    x = np.asarray(x, np.float32)
    s1 = (bn1_gamma / np.sqrt(bn1_var + EPS)).astype(np.float32)
    t1 = (bn1_beta - bn1_mean * s1 + s1 * float(np.asarray(b_dw).reshape(-1)[0])).astype(np.float32)
    s2 = (bn2_gamma / np.sqrt(bn2_var + EPS)).astype(np.float32)
    t2 = (bn2_beta - bn2_mean * s2 + s2 * np.asarray(b_pw, np.float32)).astype(np.float32)

    # s1 applied during the transpose matmul (block-diag rhs);
    # s2 folded into the pointwise weights (inside relu arg, sign-free).
    sdiag = np.zeros((128, CT * 128), np.float32)
    for ci in range(CT):
        sdiag[:, 128 * ci:128 * (ci + 1)] = np.diag(s1[128 * ci:128 * (ci + 1)])
    wS = np.asarray(w_pw, np.float32) * s2[:, None]          # [CO, C]

    shared = {
        "bmat": _build_bmats(np.asarray(w_dw, np.float32)[0, 0]).astype(bf),
        "sdiag": sdiag.astype(bf),
        "wT": np.ascontiguousarray(wS.T).astype(bf),
        "t1": np.ascontiguousarray(t1.reshape(CT, 128).T),
        "t2": np.ascontiguousarray(t2.reshape(OT, 128).T),
    }
    return [{"x": np.ascontiguousarray(x[NPC * i: NPC * (i + 1)]), **shared}
            for i in range(N_CORES)]


def kernel(x, w_dw, b_dw, bn1_gamma, bn1_beta, bn1_mean, bn1_var,
           w_pw, b_pw, bn2_gamma, bn2_beta, bn2_mean, bn2_var):
    if "nc" not in _cache:
        _cache["nc"] = _build_program()
    nc = _cache["nc"]

    in_maps = _prep_inputs(x, w_dw, b_dw, bn1_gamma, bn1_beta, bn1_mean, bn1_var,
                           w_pw, b_pw, bn2_gamma, bn2_beta, bn2_mean, bn2_var)
    try:
        if "runner" not in _cache:
            _cache["runner"] = _get_runner(nc)
        fn, in_names, out_names, out_shapes, out_dtypes = _cache["runner"]
        concat_in = [np.concatenate([m[name] for m in in_maps], axis=0)
                     for name in in_names]
        zeros = [np.zeros(s, d) for s, d in zip(out_shapes, out_dtypes)]
        outs = fn(*concat_in, *zeros)
        z = np.asarray(outs[out_names.index("z")])
        return z.astype(np.float32)
    except Exception:
        from concourse import bass_utils
        res = bass_utils.run_bass_kernel_spmd(nc, in_maps, core_ids=list(range(N_CORES)))
        z = np.concatenate([res.results[i]["z"] for i in range(N_CORES)], axis=0)
        return np.asarray(z, np.float32)


def _kernel_spmd_reference_path(x, w_dw, b_dw, bn1_gamma, bn1_beta, bn1_mean, bn1_var,
                                w_pw, b_pw, bn2_gamma, bn2_beta, bn2_mean, bn2_var):
    """Original path through bass_utils.run_bass_kernel_spmd (kept for checking)."""
    from concourse import bass_utils

    if "nc" not in _cache:
        _cache["nc"] = _build_program()
    nc = _cache["nc"]

    in_maps = _prep_inputs(x, w_dw, b_dw, bn1_gamma, bn1_beta, bn1_mean, bn1_var,
                           w_pw, b_pw, bn2_gamma, bn2_beta, bn2_mean, bn2_var)
    res = bass_utils.run_bass_kernel_spmd(nc, in_maps, core_ids=list(range(N_CORES)))
    z = np.concatenate([res.results[i]["z"] for i in range(N_CORES)], axis=0)
    return np.asarray(z, np.float32)


# revision 22
# speedup vs baseline: 257.2367x; 1.0005x over previous
"""Trainium2 Bass kernel for DepthWiseSeparableConv (shared-3x3 dw conv + BN+ReLU + 1x1 conv + BN+ReLU).

Strategy (8 NeuronCores, data-parallel over batch N=32 -> 4 images/core):
  - Load x [c,hw] (cast fp32->bf16 during SWDGE DMA).
  - PE transpose (regular matmul) to x^T chunks [112 pix, c]; the moving
    operand is block-diag(s1) so BN1's scale is applied for free here.
  - Depthwise 3x3 conv as banded matmuls, scatter form: for each source
    chunk (stationary = x^T chunk), matmuls against [B+1|B0|B-1] slices
    accumulate into PSUM pair tiles -> output lands directly in [c, pix].
  - BN1 shift + ReLU: one op (bias-add + max0), alternating ScalarE/VectorE,
    cast to bf16 -> y [c, pix].
  - 1x1 conv = GEMM over 2 c-tiles; BN2's scale is folded into the weights
    (inside the relu argument, so no sign assumption), shift+ReLU as 1 op.
  - z fp32 -> DMA out per 448-pixel block.

Self-contained: hardcodes all shapes; no file reads.
"""

import numpy as np

N, C, CO, H, W = 32, 256, 512, 56, 56
EPS = 1e-5
N_CORES = 8
NPC = N // N_CORES      # images per core
HW = H * W              # 3136
CH = 112                # pixel chunk = 2 rows of 56
NCHUNK = HW // CH       # 28
NPAIR = NCHUNK // 2     # 14
CT = C // 128           # 2 c-tiles
OT = CO // 128          # 4 o-tiles
GN = 448                # gemm pixel-block
NPB = HW // GN          # 7

_cache = {}


def _build_program():
    import concourse.mybir as mybir
    import concourse.tile as tile
    from concourse import bacc

    f32 = mybir.dt.float32
    bf16 = mybir.dt.bfloat16

    nc = bacc.Bacc("TRN2", target_bir_lowering=False, debug=False)

    x_d = nc.dram_tensor("x", [NPC, C, H, W], f32, kind="ExternalInput").ap()
    bmat_d = nc.dram_tensor("bmat", [CH, 3 * CH], bf16, kind="ExternalInput").ap()
    # block-diagonal scale: sdiag[:, ci*128:(ci+1)*128] = diag(s1[ci-tile])
    sdiag_d = nc.dram_tensor("sdiag", [128, CT * 128], bf16, kind="ExternalInput").ap()
    wT_d = nc.dram_tensor("wT", [C, CO], bf16, kind="ExternalInput").ap()  # s2-folded
    t1_d = nc.dram_tensor("t1", [128, CT], f32, kind="ExternalInput").ap()
    t2_d = nc.dram_tensor("t2", [128, OT], f32, kind="ExternalInput").ap()
    z_d = nc.dram_tensor("z", [NPC, CO, H, W], f32, kind="ExternalOutput").ap()

    relu = mybir.ActivationFunctionType.Relu
    add = mybir.AluOpType.add
    amax = mybir.AluOpType.max

    with tile.TileContext(nc) as tc:
        with (
            tc.tile_pool(name="singles", bufs=1) as singles,
            tc.tile_pool(name="xp", bufs=3) as xp,
            tc.tile_pool(name="xtp", bufs=2) as xtp,
            tc.tile_pool(name="yp", bufs=2) as yp,
            tc.tile_pool(name="zp", bufs=8) as zp,
            tc.tile_pool(name="tps", bufs=2, space="PSUM") as tps,
            tc.tile_pool(name="cvs", bufs=2, space="PSUM") as cvs,
            tc.tile_pool(name="zps", bufs=2, space="PSUM") as zps,
        ):
            bmat_sb = singles.tile([CH, 3 * CH], bf16)
            nc.sync.dma_start(out=bmat_sb, in_=bmat_d)
            sdiag_sb = singles.tile([128, CT * 128], bf16)
            nc.sync.dma_start(out=sdiag_sb, in_=sdiag_d)
            w_sb = singles.tile([128, CT, CO], bf16)
            for ci in range(CT):
                nc.sync.dma_start(out=w_sb[:, ci, :], in_=wT_d[128 * ci:128 * (ci + 1), :])
            t1_sb = singles.tile([128, CT], f32)
            nc.sync.dma_start(out=t1_sb, in_=t1_d)
            t2_sb = singles.tile([128, OT], f32)
            nc.sync.dma_start(out=t2_sb, in_=t2_d)

            # epilogue helper: out = relu(in + bias[p]) on alternating engines
            epi_ctr = [0]

            def epilogue(out_ap, in_ap, bias_ap):
                use_act = (epi_ctr[0] % 2 == 0)
                epi_ctr[0] += 1
                if bias_ap is None:
                    if use_act:
                        nc.scalar.copy(out=out_ap, in_=in_ap)
                    else:
                        nc.vector.tensor_copy(out_ap, in_ap)
                elif use_act:
                    nc.scalar.activation(out=out_ap, in_=in_ap, func=relu,
                                         bias=bias_ap, scale=1.0)
                else:
                    nc.vector.tensor_scalar(out=out_ap, in0=in_ap,
                                            scalar1=bias_ap, scalar2=0.0,
                                            op0=add, op1=amax)

            # conv moving-operand slices of bmat = [B+1 | B0 | B-1]
            A_even = bmat_sb[:, CH:3 * CH]       # [B0 | B-1]
            A_odd = bmat_sb[:, 0:2 * CH]         # [B+1 | B0]
            B_plus = bmat_sb[:, 0:CH]            # B+1 (from even source s -> chunk s-1)
            B_minus = bmat_sb[:, 2 * CH:3 * CH]  # B-1 (from odd source s -> chunk s+1)

            for img in range(NPC):
                # ---- stage A: load x, cast fp32->bf16 during DMA ----
                # 16 pad cols so transpose stationaries can read 128 cols (FWL)
                x_sb = xp.tile([128, CT, HW + 16], bf16, tag="x")
                for ci in range(CT):
                    nc.vector.memset(x_sb[:, ci, HW:], 0.0)
                    xflat = x_d[img, 128 * ci:128 * (ci + 1), :, :].rearrange("c h w -> c (h w)")
                    for hh in range(2):  # halves so transposes start sooner
                        nc.gpsimd.dma_start(
                            out=x_sb[:, ci, HW // 2 * hh: HW // 2 * (hh + 1)],
                            in_=xflat[:, HW // 2 * hh: HW // 2 * (hh + 1)],
                        )

                # ---- stage B: transpose (x^T scaled by s1 via block-diag rhs) ----
                xt_sb = xtp.tile([CH, NCHUNK * C], bf16, tag="xt")
                for q in range(NPAIR):
                    # stationary reads 128 cols (112 real + 16 overlap/pad) so
                    # FWL engages; psum rows 112:128 are written but never read
                    t_ps = tps.tile([128, 512], f32, tag="tps")
                    for k in range(2):          # chunk s = 2q+k
                        s = 2 * q + k
                        for ci in range(CT):
                            nc.tensor.matmul(
                                t_ps[:, 256 * k + 128 * ci: 256 * k + 128 * (ci + 1)],
                                lhsT=x_sb[:, ci, CH * s: CH * s + 128],
                                rhs=sdiag_sb[:, 128 * ci:128 * (ci + 1)],
                                start=True, stop=True,
                            )
                    epilogue(xt_sb[:, 512 * q: 512 * (q + 1)], t_ps[0:CH, :], None)

                def xt(s, ci):
                    return xt_sb[:, 256 * s + 128 * ci: 256 * s + 128 * (ci + 1)]

                # ---- stage C: depthwise conv, scatter form over sources ----
                # combined psum tile g covers chunks 4g..4g+3 as [128, bank 2, 256]
                y_sb = yp.tile([128, CT, HW], bf16, tag="y")
                NG = NCHUNK // 4  # 7
                for ci in range(CT):
                    for g in range(NG):
                        # each [*, q, :] half is a full 2 KiB PSUM bank (512 f32);
                        # only cols 0:224 are used. Separate banks so per-bank
                        # start=True clears cannot disturb the sibling pair.
                        cv = cvs.tile([128, 2, 512], f32, tag="cv")
                        mms = []  # (src_chunk, rhs, bank, lo, hi)
                        if g > 0:
                            mms.append((4 * g - 1, B_minus, 0, 0, CH))
                        mms.append((4 * g, A_even, 0, 0, 2 * CH))
                        mms.append((4 * g + 1, A_odd, 0, 0, 2 * CH))
                        mms.append((4 * g + 1, B_minus, 1, 0, CH))
                        mms.append((4 * g + 2, A_even, 1, 0, 2 * CH))
                        mms.append((4 * g + 2, B_plus, 0, CH, 2 * CH))
                        mms.append((4 * g + 3, A_odd, 1, 0, 2 * CH))
                        if g < NG - 1:
                            mms.append((4 * g + 4, B_plus, 1, CH, 2 * CH))
                        first = {0: True, 1: True}
                        last_idx = {0: -1, 1: -1}
                        for i, (_, _, bank, _, _) in enumerate(mms):
                            last_idx[bank] = i
                        for i, (s, rhs, bank, lo, hi) in enumerate(mms):
                            nc.tensor.matmul(
                                cv[:, bank, lo:hi], lhsT=xt(s, ci), rhs=rhs,
                                start=first[bank], stop=(i == last_idx[bank]),
                                skip_group_check=True,
                            )
                            first[bank] = False
                        epilogue(
                            y_sb[:, ci, 448 * g: 448 * (g + 1)].rearrange(
                                "p (a b) -> p a b", a=2),
                            cv[:, :, 0:2 * CH],
                            t1_sb[:, ci:ci + 1],
                        )

                # ---- stage D: pointwise GEMM + BN2+ReLU (s2 folded into W) ----
                for oi in range(OT):
                    for pb in range(NPB):
                        z_ps = zps.tile([128, GN], f32, tag="zps")
                        for ci in range(CT):
                            nc.tensor.matmul(
                                z_ps,
                                lhsT=w_sb[:, ci, 128 * oi:128 * (oi + 1)],
                                rhs=y_sb[:, ci, GN * pb: GN * (pb + 1)],
                                start=(ci == 0), stop=(ci == CT - 1),
                            )
                        z_sb = zp.tile([128, GN], f32, tag="z")
                        epilogue(z_sb, z_ps, t2_sb[:, oi:oi + 1])
                        # ---- stage E: DMA out per pixel-block (finer overlap) ----
                        nc.sync.dma_start(
                            out=z_d[img, 128 * oi:128 * (oi + 1), :, :]
                                .rearrange("o h w -> o (h w)")[:, GN * pb: GN * (pb + 1)],
                            in_=z_sb,
                        )

    nc.compile()
    return nc


def _build_bmats(k2d):
    """B[t][p,j] = k2d[1+dh, 1+dw] with dh = 2t + p//56 - j//56, dw = p%56 - j%56."""
    p = np.arange(CH)
    j = np.arange(CH)
    ph, pw = p // W, p % W
    jh, jw = j // W, j % W
    out = []
    for t in (1, 0, -1):  # concat order [B_{+1} | B_0 | B_{-1}]
        dh = 2 * t + ph[:, None] - jh[None, :]
        dw = pw[:, None] - jw[None, :]
        ok = (np.abs(dh) <= 1) & (np.abs(dw) <= 1)
        B = np.where(ok, k2d[np.clip(1 + dh, 0, 2), np.clip(1 + dw, 0, 2)], 0.0)
        out.append(B)
    return np.concatenate(out, axis=1).astype(np.float32)  # [112, 336]


def _get_runner(nc):
    """Build a cached jitted shard_map executable mirroring
    concourse.bass2jax.run_bass_via_pjrt (which re-traces on every call)."""
    import jax
    import jax.numpy as jnp
    import concourse.mybir as mybir
    from jax.sharding import Mesh, PartitionSpec
    from jax.experimental.shard_map import shard_map
    from concourse.bass2jax import (
        _bass_exec_p, install_neuronx_cc_hook, partition_id_tensor)

    install_neuronx_cc_hook()

    partition_name = nc.partition_id_tensor.name if nc.partition_id_tensor else None

    in_names, out_names, out_avals = [], [], []
    for alloc in nc.m.functions[0].allocations:
        if not isinstance(alloc, mybir.MemoryLocationSet):
            continue
        name = alloc.memorylocations[0].name
        if alloc.kind == "ExternalInput":
            if name != partition_name:
                in_names.append(name)
        elif alloc.kind == "ExternalOutput":
            out_names.append(name)
            out_avals.append(jax.core.ShapedArray(
                tuple(alloc.tensor_shape), mybir.dt.np(alloc.dtype)))
    n_params = len(in_names)
    all_names = list(in_names) + list(out_names)
    if partition_name is not None:
        all_names.append(partition_name)

    def _body(*args):
        operands = list(args)
        if partition_name is not None:
            operands.append(partition_id_tensor())
        return tuple(_bass_exec_p.bind(
            *operands,
            out_avals=tuple(out_avals),
            in_names=tuple(all_names),
            out_names=tuple(out_names),
            lowering_input_output_aliases=(),
            sim_require_finite=True,
            sim_require_nnan=True,
            nc=nc,
        ))

    n_outs = len(out_avals)
    devices = jax.devices()[:N_CORES]
    mesh = Mesh(np.asarray(devices), ("core",))
    fn = jax.jit(
        shard_map(
            _body, mesh=mesh,
            in_specs=(PartitionSpec("core"),) * (n_params + n_outs),
            out_specs=(PartitionSpec("core"),) * len(out_names),
            check_rep=False,
        ),
        donate_argnums=tuple(range(n_params, n_params + n_outs)),
        keep_unused=True,
    )
    out_shapes = [(N_CORES * a.shape[0], *a.shape[1:]) for a in out_avals]
    out_dtypes = [a.dtype for a in out_avals]
    return fn, in_names, out_names, out_shapes, out_dtypes


def _prep_inputs(x, w_dw, b_dw, bn1_gamma, bn1_beta, bn1_mean, bn1_var,
                 w_pw, b_pw, bn2_gamma, bn2_beta, bn2_mean, bn2_var):
    import ml_dtypes
    bf = ml_dtypes.bfloat16

    x = np.asarray(x, np.float32)
    s1 = (bn1_gamma / np.sqrt(bn1_var + EPS)).astype(np.float32)
    t1 = (bn1_beta - bn1_mean * s1 + s1 * float(np.asarray(b_dw).reshape(-1)[0])).astype(np.float32)
    s2 = (bn2_gamma / np.sqrt(bn2_var + EPS)).astype(np.float32)
    t2 = (bn2_beta - bn2_mean * s2 + s2 * np.asarray(b_pw, np.float32)).astype(np.float32)

    # s1 applied during the transpose matmul (block-diag rhs);
    # s2 folded into the pointwise weights (inside relu arg, sign-free).
    sdiag = np.zeros((128, CT * 128), np.float32)
    for ci in range(CT):
        sdiag[:, 128 * ci:128 * (ci + 1)] = np.diag(s1[128 * ci:128 * (ci + 1)])
    wS = np.asarray(w_pw, np.float32) * s2[:, None]          # [CO, C]

    shared = {
        "bmat": _build_bmats(np.asarray(w_dw, np.float32)[0, 0]).astype(bf),
        "sdiag": sdiag.astype(bf),
        "wT": np.ascontiguousarray(wS.T).astype(bf),
        "t1": np.ascontiguousarray(t1.reshape(CT, 128).T),
        "t2": np.ascontiguousarray(t2.reshape(OT, 128).T),
    }
    return [{"x": np.ascontiguousarray(x[NPC * i: NPC * (i + 1)]), **shared}
            for i in range(N_CORES)]


def kernel(x, w_dw, b_dw, bn1_gamma, bn1_beta, bn1_mean, bn1_var,
           w_pw, b_pw, bn2_gamma, bn2_beta, bn2_mean, bn2_var):
    if "nc" not in _cache:
        _cache["nc"] = _build_program()
    nc = _cache["nc"]

    in_maps = _prep_inputs(x, w_dw, b_dw, bn1_gamma, bn1_beta, bn1_mean, bn1_var,
                           w_pw, b_pw, bn2_gamma, bn2_beta, bn2_mean, bn2_var)
    try:
        # cached-jit PJRT path only under axon (native NRT boxes take the
        # run_bass_kernel_spmd path below, which drives /dev/neuron* directly)
        from concourse._compat import axon_active
        if not axon_active():
            raise RuntimeError("native NRT environment")
        if "runner" not in _cache:
            _cache["runner"] = _get_runner(nc)
        fn, in_names, out_names, out_shapes, out_dtypes = _cache["runner"]
        concat_in = [np.concatenate([m[name] for m in in_maps], axis=0)
                     for name in in_names]
        zeros = [np.zeros(s, d) for s, d in zip(out_shapes, out_dtypes)]
        outs = fn(*concat_in, *zeros)
        z = np.asarray(outs[out_names.index("z")])
        return z.astype(np.float32)
    except Exception:
        from concourse import bass_utils
        res = bass_utils.run_bass_kernel_spmd(nc, in_maps, core_ids=list(range(N_CORES)))
        z = np.concatenate([res.results[i]["z"] for i in range(N_CORES)], axis=0)
        return np.asarray(z, np.float32)


def _kernel_spmd_reference_path(x, w_dw, b_dw, bn1_gamma, bn1_beta, bn1_mean, bn1_var,
                                w_pw, b_pw, bn2_gamma, bn2_beta, bn2_mean, bn2_var):
    """Original path through bass_utils.run_bass_kernel_spmd (kept for checking)."""
    from concourse import bass_utils

    if "nc" not in _cache:
        _cache["nc"] = _build_program()
    nc = _cache["nc"]

    in_maps = _prep_inputs(x, w_dw, b_dw, bn1_gamma, bn1_beta, bn1_mean, bn1_var,
                           w_pw, b_pw, bn2_gamma, bn2_beta, bn2_mean, bn2_var)
    res = bass_utils.run_bass_kernel_spmd(nc, in_maps, core_ids=list(range(N_CORES)))
    z = np.concatenate([res.results[i]["z"] for i in range(N_CORES)], axis=0)
    return np.asarray(z, np.float32)
